# revision 42
# baseline (speedup 1.0000x reference)
"""Multi-head attention (B=4, T=2048, D=1024, H=16) on 8 TRN2 NeuronCores.

Sharding: core c -> (batch b = c//2, head-group g = c%2 of 8 heads).
Each core computes the qkv projection for its batch restricted to its 8
heads, full attention for those heads, and a partial output projection
(ctx_local @ Wout[rows of its heads]).  Host sums the two partials per batch.

All device inputs are pre-cast to bf16 on the host.  Per-core kernel,
organized so the PE stream is dense and ACT (softmax exp) saturated:

  x kept fully resident in SBUF (loaded once, 4 early DMAs on separate
  engine queues, ordered by first use: x-span0, wq, wk, rest of x, wv,
  wout last);
  qk-projection for head-pair 0, then v-projection (all heads),
  then for each head-pair hc: attention for both heads over all query
  quarters (S pairs = two row-tiled concurrent matmuls, one per head;
  exp on ACT [128,1024] PSUM->SBUF; ctx^T+sumexp via [v|1].T @ P;
  normalization via DVE reciprocal + gpsimd partition broadcast),
  interleaved with the qk-projection of the next pair; during the last
  pair, the output projection runs per query quarter.

PSUM: spsum 2x2 banks, ctx ring 3x1 banks (so the norm chain of quarter
q never blocks quarter q+1's AV), proj 1 bank.  P2 (exp output) is a
4-chunk ring (AV trails exp by one chunk).
"""

import numpy as np
import ml_dtypes
from contextlib import ExitStack

import concourse.bass as bass
import concourse.bacc as bacc
import concourse.tile as tile
from concourse import mybir
from concourse.bass_utils import run_bass_kernel_spmd
from concourse.tile_rust import add_dep_helper

FP32 = mybir.dt.float32
BF16 = mybir.dt.bfloat16
EXP = mybir.ActivationFunctionType.Exp

D = 1024
T = 2048
HPC = 8          # heads per core
FC = 8           # feature chunks of 128 (projection contraction)
TS = 4           # token spans of 512
KC = 16          # k chunks of 128
QQ = 4           # query quarters of 512
PR = 8           # P2 ring depth (chunks)

# Within-block AV emission schedule: AV for chunk kc is deferred ~4 chunks so
# the PE never waits on exp, then catches up at the block end so AV(15) (and
# with it the softmax-normalization chain) starts right after exp(15) — the
# next block's first S/exp pair follows without an ACT gap, and by the time
# its head-B AV needs the psum ctx slot the previous norm has released it.
AV_DUE = {4: (0,), 5: (1,), 6: (2,), 7: (3,), 8: (4,), 9: (5, 6), 10: (7,),
          11: (8, 9), 12: (10,), 13: (11, 12), 14: (13,), 15: (14,)}


def _norm(nc, rpool, ctx_sb, ctxp, hh, hc, qsl):
    """ctx_sb[hb:hb+64, hc, qsl] = ctxp[0:64] / ctxp[64] (sumexp row)."""
    hb = (hh % 2) * 64
    rtmp = rpool.tile([1, 512], FP32, tag="rtmp")
    nc.vector.tensor_copy(out=rtmp[:], in_=ctxp[64:65, :])
    rt = rpool.tile([1, 512], FP32, tag="rt")
    nc.vector.reciprocal_approx_fast(out=rt[:], in_=rtmp[:])
    rb = rpool.tile([64, 512], FP32, tag="rb")
    nc.gpsimd.partition_broadcast(rb[:], rt[0:1, :], channels=64)
    nc.vector.tensor_mul(ctx_sb[hb:hb + 64, hc, qsl], ctxp[0:64, :], rb[:])


def _qk_proj(nc, ps, x_sb, wq_sb, wk_sb, qT, kT, hc, ts_list=range(TS)):
    """qT/kT[:, hc, :] for head pair hc: out [dims 128, tok 512] per span."""
    for ts in ts_list:
        tsl = slice(ts * 512, (ts + 1) * 512)
        for w_sb, dst in ((wq_sb, qT), (wk_sb, kT)):
            p = ps.tile([128, 512], FP32, tag="proj")
            for fc in range(FC):
                nc.tensor.matmul(
                    p[:],
                    lhsT=w_sb[:, fc, hc * 128:(hc + 1) * 128],
                    rhs=x_sb[:, fc, tsl],
                    start=(fc == 0), stop=(fc == FC - 1))
            nc.vector.tensor_copy(out=dst[:, hc, tsl], in_=p[:])


def _attention(nc, ppool, spsum, cpsum, rpool, qT, kT, v_sb, ctx_sb, hc, qq,
               extra=None, av_due=AV_DUE, post_av=()):
    """Both heads of pair hc for query quarter qq.

    ``extra`` (called once per k-chunk) interleaves other PE work (the fused
    v-projection, the next pair's qk-projection, the output projection) into
    the ACT-bound attention stream."""
    qsl = slice(qq * 512, (qq + 1) * 512)
    P2 = ppool.tile([128, PR, 2, 512], BF16, tag="P2")
    ctxA = cpsum.tile([65, 512], FP32, tag="ctx")
    ctxB = cpsum.tile([65, 512], FP32, tag="ctx")
    def emit_av(kc):
        for i, ctxp in ((0, ctxA), (1, ctxB)):
            nc.tensor.matmul(
                ctxp[:],
                lhsT=v_sb[:, kc, 2 * hc + i, :],
                rhs=P2[:, kc % PR, i, :],
                start=(kc == 0), stop=(kc == KC - 1))

    for kc in range(KC):
        sps = spsum.tile([128, 2, 512], FP32, tag="S")
        for i in range(2):          # head A on rows 0-63, head B on 64-127
            b0 = i * 64
            nc.tensor.matmul(
                sps[:, i, :],
                lhsT=kT[b0:b0 + 64, hc, kc * 128:(kc + 1) * 128],
                rhs=qT[b0:b0 + 64, hc, qsl],
                start=True, stop=True)
        nc.scalar.activation(
            out=P2[:, kc % PR, :, :], in_=sps[:, :, :], func=EXP, scale=0.125)
        if extra is not None:
            extra(kc)
        for av_kc in av_due.get(kc, ()):
            emit_av(av_kc)
    for av_kc in post_av:
        emit_av(av_kc)
    emit_av(KC - 1)
    _norm(nc, rpool, ctx_sb, ctxA, 2 * hc, hc, qsl)
    _norm(nc, rpool, ctx_sb, ctxB, 2 * hc + 1, hc, qsl)


def _body(ctx, nc, tc, xt_d, wq_d, wk_d, wv_d, wo_d, out_d):
    xt_r = xt_d.rearrange("(f p) t -> p f t", p=128)
    persist = ctx.enter_context(tc.tile_pool(name="persist", bufs=1))
    qT = persist.tile([128, 4, T], BF16, tag="qT")
    kT = persist.tile([128, 4, T], BF16, tag="kT")
    v_sb = persist.tile([128, KC, HPC, 65], BF16, tag="v")
    ctx_sb = persist.tile([128, 4, T], BF16, tag="ctx")
    wo_sb = persist.tile([128, 4, D], BF16, tag="wo")
    x_sb = persist.tile([128, FC, T], BF16, tag="x")
    wq_sb = persist.tile([128, FC, 512], BF16, tag="wq")
    wk_sb = persist.tile([128, FC, 512], BF16, tag="wk")
    wv_sb = persist.tile([128, FC, 512], BF16, tag="wv")

    # DMA schedule: the three DMA-capable engine queues share HBM bandwidth,
    # so each load is held (via explicit dependency edges) until the loads it
    # would otherwise steal bandwidth from are done.  Arrival order tracks
    # consumption order: x span 0 + wq + wk gate the first projection span,
    # x1/x2 the next two, x3 the last, wv the fused v-projection at the start
    # of attention, wout only head-pair 3.
    wq_r = wq_d.rearrange("(f p) c -> p f c", p=128)
    i_x0a = nc.sync.dma_start(out=x_sb[:, 0:4, 0:512], in_=xt_r[:, 0:4, 0:512])
    i_wqa = nc.scalar.dma_start(out=wq_sb[:, 0:4, :], in_=wq_r[:, 0:4, :])
    wk_r = wk_d.rearrange("(f p) c -> p f c", p=128)
    i_wka = nc.gpsimd.dma_start(out=wk_sb[:, 0:4, :], in_=wk_r[:, 0:4, :])
    i_x0b = nc.sync.dma_start(out=x_sb[:, 4:8, 0:512], in_=xt_r[:, 4:8, 0:512])
    i_wqb = nc.scalar.dma_start(out=wq_sb[:, 4:8, :], in_=wq_r[:, 4:8, :])
    i_wkb = nc.gpsimd.dma_start(out=wk_sb[:, 4:8, :], in_=wk_r[:, 4:8, :])
    i_x1 = nc.sync.dma_start(out=x_sb[:, :, 512:1024], in_=xt_r[:, :, 512:1024])
    i_x2 = nc.scalar.dma_start(out=x_sb[:, :, 1024:1536], in_=xt_r[:, :, 1024:1536])
    i_x3 = nc.gpsimd.dma_start(out=x_sb[:, :, 1536:2048], in_=xt_r[:, :, 1536:2048])
    i_wv = nc.sync.dma_start(out=wv_sb[:], in_=wv_d.rearrange("(f p) c -> p f c", p=128))
    i_wo = nc.scalar.dma_start(out=wo_sb[:], in_=wo_d.rearrange("(c p) d -> p c d", p=128))
    for a, bs in ((i_x1, (i_x0b, i_wqb)), (i_x2, (i_x0b, i_wqb, i_wkb)),
                  (i_wv, (i_x2,)), (i_x3, (i_wv,)), (i_wo, (i_x3,))):
        for b in bs:
            add_dep_helper(a.ins, b.ins, reason="dma bandwidth staging")

    nc.vector.memset(v_sb[:, :, :, 64:65], 1.0)

    # Pre-attention projections, emitted in DMA-arrival order (span by span)
    # so the PE never sits on a not-yet-loaded span: pair 0 AND pair 1 for
    # spans 0-1 (the PE would otherwise idle waiting for x spans), pair 0
    # only for spans 2-3 so attention starts as soon as x3 lands.  Pair 1's
    # spans 2-3 are interleaved into the attention stream later.
    with tc.tile_pool(name="proj0", bufs=2, space="PSUM") as ps0:
        for ts in (0, 1):
            _qk_proj(nc, ps0, x_sb, wq_sb, wk_sb, qT, kT, 0, ts_list=[ts])
            _qk_proj(nc, ps0, x_sb, wq_sb, wk_sb, qT, kT, 1, ts_list=[ts])
        # pair-0 spans 2-3 are folded into the first attention block (the PE
        # would otherwise idle there waiting for the x2/x3 DMAs)

    osb = ctx.enter_context(tc.tile_pool(name="osb", bufs=2))
    with tc.tile_pool(name="P", bufs=2) as ppool, \
         tc.tile_pool(name="proj", bufs=1, space="PSUM") as ps, \
         tc.tile_pool(name="spsum", bufs=2, space="PSUM") as spsum, \
         tc.tile_pool(name="cpsum", bufs=3, space="PSUM") as cpsum, \
         tc.tile_pool(name="rpool", bufs=2) as rpool:

        def vproj(kc):
            psv = ps.tile([128, 512], FP32, tag="proj")
            for fc in range(FC):
                nc.tensor.matmul(
                    psv[:],
                    lhsT=x_sb[:, fc, kc * 128:(kc + 1) * 128],
                    rhs=wv_sb[:, fc, :],
                    start=(fc == 0), stop=(fc == FC - 1))
            nc.vector.tensor_copy(
                out=v_sb[:, kc, :, 0:64],
                in_=psv[:].rearrange("p (h d) -> p h d", h=HPC))

        def make_qk_steps(next_hc):
            """64 generator steps: one fc-accumulation matmul per step of the
            next pair's qk projection (4 spans x {q,k} x 8 fc)."""
            st = {"p": None}

            def step(s):
                unit, fc = divmod(s, FC)
                ts, qk = divmod(unit, 2)
                tsl = slice(ts * 512, (ts + 1) * 512)
                w_sb, dst = ((wq_sb, qT), (wk_sb, kT))[qk]
                if fc == 0:
                    st["p"] = ps.tile([128, 512], FP32, tag="proj", name="qkp")
                nc.tensor.matmul(
                    st["p"][:],
                    lhsT=w_sb[:, fc, next_hc * 128:(next_hc + 1) * 128],
                    rhs=x_sb[:, fc, tsl],
                    start=(fc == 0), stop=(fc == FC - 1))
                if fc == FC - 1:
                    nc.vector.tensor_copy(out=dst[:, next_hc, tsl], in_=st["p"][:])
            return step

        def make_op_steps(qq_prev, pool=None):
            """16 steps emitting the output projection of qq_prev's tokens
            (4 token chunks x 2 column halves x accumulate 4 cc)."""
            st = {"po": None, "ot": None}
            pp = pool if pool is not None else ps

            def step(s):
                unit, half = divmod(s, 2)
                tcg = qq_prev * 4 + unit // 2
                j2 = unit % 2
                if half == 0:
                    if j2 == 0:
                        st["ot"] = osb.tile([128, D], BF16, tag="ot", name="ot")
                    st["po"] = pp.tile([128, 512], FP32, tag="proj", name="po")
                    ccs = (0, 1)
                else:
                    ccs = (2, 3)
                for cc in ccs:
                    nc.tensor.matmul(
                        st["po"][:],
                        lhsT=ctx_sb[:, cc, tcg * 128:(tcg + 1) * 128],
                        rhs=wo_sb[:, cc, j2 * 512:(j2 + 1) * 512],
                        start=(cc == 0), stop=(cc == 3))
                if half == 1:
                    nc.vector.tensor_copy(
                        out=st["ot"][:, j2 * 512:(j2 + 1) * 512], in_=st["po"][:])
                    if j2 == 1:
                        nc.sync.dma_start(
                            out=out_d[tcg * 128:(tcg + 1) * 128, :],
                            in_=st["ot"][:])
            return step

        def make_budget_extra(step_fn, n_steps, n_kc, skip_last=0):
            """Spread ``n_steps`` step_fn calls evenly over ``n_kc`` kc
            iterations (holding back ``skip_last`` steps for the caller)."""
            st = {"done": 0, "kc_seen": 0}

            def extra(kc):
                st["kc_seen"] += 1
                target = min(n_steps - skip_last,
                             (st["kc_seen"] * n_steps + n_kc - 1) // n_kc)
                while st["done"] < target:
                    step_fn(st["done"])
                    st["done"] += 1
            return extra, st

        # hc0-qq0 carries pair-0's spans 2-3 (x2/x3-gated, TS23) and the full
        # v-projection (VP, wv-gated; doubled on chunks 6-9 so every v chunk
        # lands before its AV).  Its AV schedule defers up to 7 chunks (the
        # P2 ring limit) to match the v-chunk arrival order.
        qk0_step = make_qk_steps(0)
        TS23 = {0: range(32, 40), 1: range(40, 48),
                4: range(48, 56), 5: range(56, 64)}
        VP = {2: (0,), 3: (1,), 6: (2, 3), 7: (4, 5), 8: (6, 7), 9: (8, 9),
              10: (10, 11), 11: (12,), 12: (13,), 13: (14,), 14: (15,)}
        AV_DUE_QQ0 = {7: (0,), 8: (1,), 9: (2,), 10: (3,), 11: (4, 5),
                      12: (6, 7), 13: (8, 9), 14: (10, 11), 15: (12, 13)}

        def qq0_extra(kc):
            for s in TS23.get(kc, ()):
                qk0_step(s)
            for j in VP.get(kc, ()):
                vproj(j)

        for hc in range(4):
            # hc0: vproj fills qq0, pair-1 qk spread over qq1-3 (48 kc).
            # hc1/2: next pair's qk spread over all four quarters (64 kc) so
            # each block's PE rate just matches the ACT (exp) rate.
            # hc3: output projection of the previous quarter, one step per kc;
            # for the last quarter 4 steps are held back and emitted after the
            # norms, so the PE has work while the final norm chain runs.
            qk_extra = qk_st = None
            if hc == 0:
                # pair 1's spans 0-1 were done pre-attention; spread the
                # remaining 32 steps (spans 2-3) over qq1-3
                qk_step = make_qk_steps(1)
                qk_extra, qk_st = make_budget_extra(
                    lambda i, f=qk_step: f(32 + i), 32, 48)
            elif hc < 3:
                qk_step = make_qk_steps(hc + 1)
                qk_extra, qk_st = make_budget_extra(qk_step, 64, 64)
            for qq in range(QQ):
                post = None
                if hc == 0 and qq == 0:
                    _attention(nc, ppool, spsum, cpsum, rpool,
                               qT, kT, v_sb, ctx_sb, 0, 0, extra=qq0_extra,
                               av_due=AV_DUE_QQ0, post_av=(14,))
                    continue
                if hc < 3:
                    extra = qk_extra
                elif qq >= 1:
                    op_step = make_op_steps(qq - 1)
                    hold = 8 if qq == 3 else 0
                    # skip the first 3 chunks: the previous quarter's norm
                    # (which this projection reads) lands ~3 chunks in
                    op_in, op_st = make_budget_extra(op_step, 16, 13,
                                                     skip_last=hold)

                    def extra(kc, op_in=op_in):
                        if kc >= 3:
                            op_in(kc)
                    if hold:
                        def post(op_step=op_step, op_st=op_st):
                            while op_st["done"] < 16:
                                op_step(op_st["done"])
                                op_st["done"] += 1
                else:
                    extra = None
                _attention(nc, ppool, spsum, cpsum, rpool,
                           qT, kT, v_sb, ctx_sb, hc, qq, extra=extra)
                if post is not None:
                    post()

    # Tail: output projection for the last quarter.  All attention psum pools
    # are closed, so 8 banks are free — one per (token chunk, column half).
    # The cc 0-2 accumulations depend only on head-pairs 0-2 (done long ago),
    # so the PE chews them while the last quarter's softmax normalization
    # (reciprocal/broadcast/multiply) finishes; only the cc-3 closers wait.
    with tc.tile_pool(name="ptail", bufs=8, space="PSUM") as ptail:
        pos = []
        for u in range(8):
            tcg, j2 = 12 + u // 2, u % 2
            po = ptail.tile([128, 512], FP32, tag="po", name="po")
            pos.append(po)
            for cc in (0, 1, 2):
                nc.tensor.matmul(
                    po[:],
                    lhsT=ctx_sb[:, cc, tcg * 128:(tcg + 1) * 128],
                    rhs=wo_sb[:, cc, j2 * 512:(j2 + 1) * 512],
                    start=(cc == 0), stop=False)
        ot = None
        dma_engs = (nc.sync, nc.scalar, nc.gpsimd, nc.sync)
        for u in range(8):
            tcg, j2 = 12 + u // 2, u % 2
            nc.tensor.matmul(
                pos[u][:],
                lhsT=ctx_sb[:, 3, tcg * 128:(tcg + 1) * 128],
                rhs=wo_sb[:, 3, j2 * 512:(j2 + 1) * 512],
                start=False, stop=True)
            if j2 == 0:
                ot = osb.tile([128, D], BF16, tag="ot", name="ot")
            nc.vector.tensor_copy(
                out=ot[:, j2 * 512:(j2 + 1) * 512], in_=pos[u][:])
            if j2 == 1:
                # spread the last output chunks over all three DMA queues so
                # the final drain overlaps instead of serializing on sync
                dma_engs[u // 2].dma_start(
                    out=out_d[tcg * 128:(tcg + 1) * 128, :], in_=ot[:])


def build():
    nc = bacc.Bacc("TRN2", target_bir_lowering=False, debug=False, num_devices=8)
    xt_d = nc.dram_tensor("xt", [D, T], BF16, kind="ExternalInput").ap()
    wq_d = nc.dram_tensor("wq", [D, 512], BF16, kind="ExternalInput").ap()
    wk_d = nc.dram_tensor("wk", [D, 512], BF16, kind="ExternalInput").ap()
    wv_d = nc.dram_tensor("wv", [D, 512], BF16, kind="ExternalInput").ap()
    wo_d = nc.dram_tensor("wout", [512, D], BF16, kind="ExternalInput").ap()
    out_d = nc.dram_tensor("out", [T, D], BF16, kind="ExternalOutput").ap()
    with tile.TileContext(nc) as tc:
        with ExitStack() as ctx:
            _body(ctx, nc, tc, xt_d, wq_d, wk_d, wv_d, wo_d, out_d)
    nc.compile()
    return nc


_nc = None


def _get_nc():
    global _nc
    if _nc is None:
        _nc = build()
    return _nc


def make_in_maps(x, Wqkv, Wout):
    bf = ml_dtypes.bfloat16
    in_maps = []
    for c in range(8):
        b, g = divmod(c, 2)
        cs = slice(g * 512, (g + 1) * 512)
        in_maps.append({
            "xt": np.ascontiguousarray(x[b].T).astype(bf),
            "wq": np.ascontiguousarray(Wqkv[:, 0 * D:1 * D][:, cs]).astype(bf),
            "wk": np.ascontiguousarray(Wqkv[:, 1 * D:2 * D][:, cs]).astype(bf),
            "wv": np.ascontiguousarray(Wqkv[:, 2 * D:3 * D][:, cs]).astype(bf),
            "wout": np.ascontiguousarray(Wout[cs, :]).astype(bf),
        })
    return in_maps


def kernel(x, Wqkv, Wout, _trace=False):
    nc = _get_nc()
    x = np.asarray(x, dtype=np.float32)
    Wqkv = np.asarray(Wqkv, dtype=np.float32)
    Wout = np.asarray(Wout, dtype=np.float32)
    in_maps = make_in_maps(x, Wqkv, Wout)
    kwargs = {}
    if _trace:
        kwargs["trace"] = True
    res = run_bass_kernel_spmd(nc, in_maps, core_ids=list(range(8)), **kwargs)
    outs = [res.results[c]["out"].astype(np.float32) for c in range(8)]
    out = np.stack([outs[2 * b] + outs[2 * b + 1] for b in range(4)])
    if _trace:
        kernel.last_result = res
    return out
